# revision 18
# baseline (speedup 1.0000x reference)
"""VP-SDE Euler-Maruyama forward diffusion on 8 Trainium2 NeuronCores.

Recurrence (per element, 100 steps):
    x_t = a_t * x_{t-1} + b_t * n_t
      a_t = 1 - 0.5 * beta_t * dt
      b_t = sqrt(beta_t * dt)
      beta_t = BETA0 + (t/S) * (BETA1 - BETA0)

Unrolled, the whole trajectory is a small matmul over the step axis:
    x_k = gamma_k * x_0 + sum_{j<k} (b_j * gamma_k / gamma_{j+1}) * n_j
with gamma_k = prod(a_0..a_{k-1}). Stacking x_0 as a 101st input row, every
output step is one column of a [101, 101] coefficient matrix applied to the
[101, E] input block, so the serial scan becomes a PE matmul and the kernel
is purely streaming.

Sharding: data-parallel over the batch dim (64 -> 8 per core). All HBM I/O
is bf16 (inputs pre-cast on host, outputs upcast on host), which halves the
~101 MiB/core of f32 traffic; the tolerance budget easily covers bf16
rounding. Memory-bound: ~53 MiB of HBM traffic per core.
"""

import os

import ml_dtypes
import numpy as np

import concourse.bass as bass
import concourse.mybir as mybir
from concourse.bass_utils import run_bass_kernel_spmd
from concourse.tile import TileContext

S = 100          # diffusion steps
N, L, D = 64, 256, 64
NCORES = 8
NB = N // NCORES           # batch per core
E = NB * L * D             # elements per core (131072)
K = S + 1                  # matmul contraction: 100 noise rows + x0 row
C = 8192                   # elements per DMA chunk (16 KiB/partition bf16)
MM = 512                   # matmul moving free size (one PSUM bank of f32)

BETA0, BETA1 = 0.1, 20.0
DT = 1.0 / S

F32 = mybir.dt.float32
BF16 = mybir.dt.bfloat16
NP_BF16 = ml_dtypes.bfloat16

LAST_EXEC_NS = None


def _coeff_matrix() -> np.ndarray:
    """W[j, k] = coefficient of input row j in output step k (fp64 -> bf16).

    Rows 0..99 are the noise steps, row 100 is x_0. Column k is output step k
    (k=0 is the initial state itself).
    """
    beta = BETA0 + (np.arange(S, dtype=np.float64) / S) * (BETA1 - BETA0)
    a = 1.0 - 0.5 * beta * DT
    b = np.sqrt(beta * DT)
    gamma = np.concatenate([[1.0], np.cumprod(a)])  # gamma[k] = prod(a[:k])
    W = np.zeros((K, K), dtype=np.float64)
    W[S, :] = gamma                                  # x0 coefficient
    for j in range(S):
        W[j, j + 1 :] = b[j] * gamma[j + 1 :] / gamma[j + 1]
    return W.astype(NP_BF16)


def _legalize_waits(nc, max_waits=1):
    """Split multi-sem waits into standalone EventSemaphore instructions.

    TRN2 TPB instruction encodings carry a single sem-wait slot; walrus
    rejects instructions with more ("Too many sync wait commands"). Tile
    emits up to 3 waits per instruction, so peel the excess onto
    same-engine EventSemaphore instructions placed immediately before —
    engine-queue program order makes this exactly equivalent.
    """
    split_types = tuple(
        t
        for t in (
            getattr(mybir, n, None)
            for n in (
                "InstTensorTensor",
                "InstActivation",
                "InstDMACopy",
                "InstTensorScalarPtr",
                "InstMemset",
                "InstTensorCopy",
                "InstTensorReduce",
                "InstCopy",
                "InstDrain",
                "InstMatmult",
                "InstLdweights",
            )
        )
        if t is not None
    )
    n = 0
    for fn in nc.m.functions:
        for blk in fn.blocks:
            out = []
            for inst in blk.instructions:
                si = inst.sync_info
                if (
                    si is not None
                    and si.on_wait
                    and len(si.on_wait) > max_waits
                    and isinstance(inst, split_types)
                ):
                    for w in si.on_wait[:-max_waits]:
                        n += 1
                        es = mybir.InstEventSemaphore(
                            name=f"legalize-wait-{n}", ins=[], outs=[]
                        )
                        es.name = f"legalize-wait-{n}"
                        es.engine = inst.engine
                        es.sync_info = mybir.SyncInfo(on_wait=[w], on_update=[])
                        nc.register_instruction(es)
                        out.append(es)
                    inst.sync_info = mybir.SyncInfo(
                        on_wait=list(si.on_wait[-max_waits:]),
                        on_update=list(si.on_update or []),
                    )
                out.append(inst)
            blk.instructions = out


def _build():
    nc = bass.Bass()
    nz = nc.declare_dram_parameter("nz", [K, E], BF16, isOutput=False)
    w = nc.declare_dram_parameter("w", [K, K], BF16, isOutput=False)
    out = nc.declare_dram_parameter("out", [K, E], BF16, isOutput=True)

    # HWDGE fans a DMA's partition lines over g SDMA engines where g is the
    # largest divisor of the partition count <= 16. 101 is prime -> g=1, the
    # whole transfer serializes on one engine at ~27 GB/s. Split every
    # 101-partition transfer into 96 + 5 rows (16 engines x 6 lines + 5
    # engines x 1 line) to keep all 16 engines fed.
    PS = 96

    # Tapered chunk schedule: small chunks at both ends shrink pipeline
    # fill/drain; big 8192-element chunks (16 KiB/partition lines) carry the
    # steady state.
    sizes = [2048, 2048, 4096] + [8192] * 14 + [4096, 2048, 2048]
    assert sum(sizes) == E

    with TileContext(nc) as tc:
        with (
            tc.tile_pool(name="wpool", bufs=1) as wpool,
            tc.tile_pool(name="npool", bufs=4) as npool,
            tc.tile_pool(name="opool", bufs=3) as opool,
            tc.tile_pool(name="pspool", bufs=8, space="PSUM") as pspool,
        ):
            wt = wpool.tile([K, K], BF16)
            nc.gpsimd.dma_start(out=wt[:], in_=w[:])
            off = 0
            for ci, cs in enumerate(sizes):
                # Alternate rings per chunk: each HWDGE ring carries a mix of
                # reads and writes, so every SDMA engine has descriptors from
                # both directions in flight (overlaps HBM read latency).
                rd = nc.sync if ci % 2 == 0 else nc.scalar
                wr = nc.scalar if ci % 2 == 0 else nc.sync
                nt = npool.tile([K, C], BF16)
                csl = slice(off, off + cs)
                rd.dma_start(out=nt[:PS, :cs], in_=nz[:PS, csl])
                nc.gpsimd.dma_start(out=nt[PS:, :cs], in_=nz[PS:, csl])
                ot = opool.tile([K, C], BF16)
                for j in range(cs // MM):
                    ps = pspool.tile([K, MM], F32)
                    nc.tensor.matmul(
                        out=ps[:],
                        lhsT=wt[:],
                        rhs=nt[:, j * MM : (j + 1) * MM],
                        start=True,
                        stop=True,
                    )
                    oslc = ot[:, j * MM : (j + 1) * MM]
                    if j % 2 == 0:
                        nc.vector.tensor_copy(oslc, ps[:])
                    else:
                        nc.scalar.copy(oslc, ps[:])
                wr.dma_start(out=out[:PS, csl], in_=ot[:PS, :cs])
                nc.gpsimd.dma_start(out=out[PS:, csl], in_=ot[PS:, :cs])
                off += cs
    _legalize_waits(nc)
    return nc


_NC = None
_W = None


def _install_trace_hook():
    """Register the axon NTFF profile hook (test-only; KERNEL_TRACE=1).

    The image's antenv package lacks axon_hooks, so run_bass_kernel_spmd's
    trace path degrades. Replicate the boot shim: drive NRT profiling via
    ctypes into libaxon_pjrt.so and seed sys.modules so bass_utils finds it.
    """
    import contextlib
    import ctypes
    import sys
    import types

    if "antenv.axon_hooks" in sys.modules:
        return
    so_path = "/opt/axon/libaxon_pjrt.so"
    lib = ctypes.CDLL(so_path)
    if not hasattr(lib, "axon_start_nrt_profile"):
        return
    lib.axon_start_nrt_profile.argtypes = [
        ctypes.POINTER(ctypes.c_int64),
        ctypes.c_size_t,
    ]
    lib.axon_start_nrt_profile.restype = ctypes.c_int64
    lib.axon_stop_nrt_profile.argtypes = [ctypes.c_char_p]
    lib.axon_stop_nrt_profile.restype = ctypes.c_int64

    @contextlib.contextmanager
    def _hook(output_dir, device_ids):
        import jax

        jax.devices()
        if device_ids:
            ids = (ctypes.c_int64 * len(device_ids))(*device_ids)
            rc = lib.axon_start_nrt_profile(ids, len(device_ids))
        else:
            rc = lib.axon_start_nrt_profile(None, 0)
        if rc != 0:
            raise RuntimeError(f"axon_start_nrt_profile rc={rc}")
        try:
            yield
        finally:
            n = lib.axon_stop_nrt_profile(str(output_dir).encode())
            print(f"profile: {n} file(s) written to {output_dir}", file=sys.stderr)

    mod = types.ModuleType("antenv.axon_hooks")
    mod.get_axon_ntff_profile_hook = lambda: _hook
    mod.set_axon_ntff_profile_hook = lambda h: None
    sys.modules["antenv.axon_hooks"] = mod

    # The trace path uploads NEFF artifacts to a remote bucket; no-op it.
    import concourse.bass_utils as _bu

    _bu.upload_artifacts = lambda tmpdir: tmpdir


def _to_bf16(arr: np.ndarray) -> np.ndarray:
    """Fast round-to-nearest-even f32 -> bf16 via integer ops."""
    u = np.ascontiguousarray(arr, dtype=np.float32).view(np.uint32)
    rounded = (u + 0x7FFF + ((u >> 16) & 1)) >> 16
    return rounded.astype(np.uint16).view(NP_BF16)


def kernel(x: np.ndarray, noise: np.ndarray) -> np.ndarray:
    global _NC, _W, LAST_EXEC_NS
    if _NC is None:
        _NC = _build()
        _W = _coeff_matrix()

    in_maps = []
    for c in range(NCORES):
        xs = _to_bf16(x[c * NB : (c + 1) * NB]).reshape(1, E)
        ns = _to_bf16(noise[:, c * NB : (c + 1) * NB]).reshape(S, E)
        in_maps.append({"nz": np.concatenate([ns, xs], axis=0), "w": _W})

    trace = bool(os.environ.get("KERNEL_TRACE"))
    if trace:
        _install_trace_hook()
    res = run_bass_kernel_spmd(_NC, in_maps, list(range(NCORES)), trace=trace)
    LAST_EXEC_NS = res.exec_time_ns

    outs = [
        res.results[c]["out"].astype(np.float32).reshape(K, NB, L, D)
        for c in range(NCORES)
    ]
    return np.concatenate(outs, axis=1)


# revision 19
# speedup vs baseline: 1.0015x; 1.0015x over previous
"""VP-SDE Euler-Maruyama forward diffusion on 8 Trainium2 NeuronCores.

Recurrence (per element, 100 steps):
    x_t = a_t * x_{t-1} + b_t * n_t
      a_t = 1 - 0.5 * beta_t * dt
      b_t = sqrt(beta_t * dt)
      beta_t = BETA0 + (t/S) * (BETA1 - BETA0)

Unrolled, the whole trajectory is a small matmul over the step axis:
    x_k = gamma_k * x_0 + sum_{j<k} (b_j * gamma_k / gamma_{j+1}) * n_j
with gamma_k = prod(a_0..a_{k-1}). Stacking x_0 as a 101st input row, every
output step is one column of a [101, 101] coefficient matrix applied to the
[101, E] input block, so the serial scan becomes a PE matmul and the kernel
is purely streaming.

Sharding: data-parallel over the batch dim (64 -> 8 per core). All HBM I/O
is bf16 (inputs pre-cast on host, outputs upcast on host), which halves the
~101 MiB/core of f32 traffic; the tolerance budget easily covers bf16
rounding. Memory-bound: ~53 MiB of HBM traffic per core.
"""

import os

import ml_dtypes
import numpy as np

import concourse.bass as bass
import concourse.mybir as mybir
from concourse.bass_utils import run_bass_kernel_spmd
from concourse.tile import TileContext

S = 100          # diffusion steps
N, L, D = 64, 256, 64
NCORES = 8
NB = N // NCORES           # batch per core
E = NB * L * D             # elements per core (131072)
K = S + 1                  # matmul contraction: 100 noise rows + x0 row
C = 8192                   # elements per DMA chunk (16 KiB/partition bf16)
MM = 512                   # matmul moving free size (one PSUM bank of f32)

BETA0, BETA1 = 0.1, 20.0
DT = 1.0 / S

F32 = mybir.dt.float32
BF16 = mybir.dt.bfloat16
NP_BF16 = ml_dtypes.bfloat16

LAST_EXEC_NS = None


def _coeff_matrix() -> np.ndarray:
    """W[j, k] = coefficient of input row j in output step k (fp64 -> bf16).

    Rows 0..99 are the noise steps, row 100 is x_0. Column k is output step k
    (k=0 is the initial state itself).
    """
    beta = BETA0 + (np.arange(S, dtype=np.float64) / S) * (BETA1 - BETA0)
    a = 1.0 - 0.5 * beta * DT
    b = np.sqrt(beta * DT)
    gamma = np.concatenate([[1.0], np.cumprod(a)])  # gamma[k] = prod(a[:k])
    W = np.zeros((K, K), dtype=np.float64)
    W[S, :] = gamma                                  # x0 coefficient
    for j in range(S):
        W[j, j + 1 :] = b[j] * gamma[j + 1 :] / gamma[j + 1]
    return W.astype(NP_BF16)


def _legalize_waits(nc, max_waits=1):
    """Split multi-sem waits into standalone EventSemaphore instructions.

    TRN2 TPB instruction encodings carry a single sem-wait slot; walrus
    rejects instructions with more ("Too many sync wait commands"). Tile
    emits up to 3 waits per instruction, so peel the excess onto
    same-engine EventSemaphore instructions placed immediately before —
    engine-queue program order makes this exactly equivalent.
    """
    split_types = tuple(
        t
        for t in (
            getattr(mybir, n, None)
            for n in (
                "InstTensorTensor",
                "InstActivation",
                "InstDMACopy",
                "InstTensorScalarPtr",
                "InstMemset",
                "InstTensorCopy",
                "InstTensorReduce",
                "InstCopy",
                "InstDrain",
                "InstMatmult",
                "InstLdweights",
            )
        )
        if t is not None
    )
    n = 0
    for fn in nc.m.functions:
        for blk in fn.blocks:
            out = []
            for inst in blk.instructions:
                si = inst.sync_info
                if (
                    si is not None
                    and si.on_wait
                    and len(si.on_wait) > max_waits
                    and isinstance(inst, split_types)
                ):
                    for w in si.on_wait[:-max_waits]:
                        n += 1
                        es = mybir.InstEventSemaphore(
                            name=f"legalize-wait-{n}", ins=[], outs=[]
                        )
                        es.name = f"legalize-wait-{n}"
                        es.engine = inst.engine
                        es.sync_info = mybir.SyncInfo(on_wait=[w], on_update=[])
                        nc.register_instruction(es)
                        out.append(es)
                    inst.sync_info = mybir.SyncInfo(
                        on_wait=list(si.on_wait[-max_waits:]),
                        on_update=list(si.on_update or []),
                    )
                out.append(inst)
            blk.instructions = out


def _build():
    nc = bass.Bass()
    nz = nc.declare_dram_parameter("nz", [K, E], BF16, isOutput=False)
    w = nc.declare_dram_parameter("w", [K, K], BF16, isOutput=False)
    out = nc.declare_dram_parameter("out", [K, E], BF16, isOutput=True)

    # HWDGE fans a DMA's partition lines over g SDMA engines where g is the
    # largest divisor of the partition count <= 16. 101 is prime -> g=1, the
    # whole transfer serializes on one engine at ~27 GB/s. Split every
    # 101-partition transfer into 96 + 5 rows (16 engines x 6 lines + 5
    # engines x 1 line) to keep all 16 engines fed.
    PS = 96

    # In-chunks use 32 KiB/partition lines (reads are HBM-latency-bound, big
    # descriptors amortize); out-chunks stay at 16 KiB lines (writes post
    # fine) and two out-DMAs per in-chunk keep the pipeline fine-grained.
    CIN = 2 * C

    with TileContext(nc) as tc:
        with (
            tc.tile_pool(name="wpool", bufs=1) as wpool,
            tc.tile_pool(name="npool", bufs=3) as npool,
            tc.tile_pool(name="opool", bufs=3) as opool,
            tc.tile_pool(name="pspool", bufs=8, space="PSUM") as pspool,
        ):
            wt = wpool.tile([K, K], BF16)
            nc.gpsimd.dma_start(out=wt[:], in_=w[:])
            for ci in range(E // CIN):
                nt = npool.tile([K, CIN], BF16)
                isl = slice(ci * CIN, (ci + 1) * CIN)
                nc.sync.dma_start(out=nt[:PS, :], in_=nz[:PS, isl])
                nc.gpsimd.dma_start(out=nt[PS:, :], in_=nz[PS:, isl])
                for h in range(2):
                    ot = opool.tile([K, C], BF16)
                    for j in range(C // MM):
                        ps = pspool.tile([K, MM], F32)
                        jj = h * (C // MM) + j
                        nc.tensor.matmul(
                            out=ps[:],
                            lhsT=wt[:],
                            rhs=nt[:, jj * MM : (jj + 1) * MM],
                            start=True,
                            stop=True,
                        )
                        oslc = ot[:, j * MM : (j + 1) * MM]
                        if j % 2 == 0:
                            nc.vector.tensor_copy(oslc, ps[:])
                        else:
                            nc.scalar.copy(oslc, ps[:])
                    csl = slice(ci * CIN + h * C, ci * CIN + (h + 1) * C)
                    nc.scalar.dma_start(out=out[:PS, csl], in_=ot[:PS, :])
                    nc.gpsimd.dma_start(out=out[PS:, csl], in_=ot[PS:, :])
    _legalize_waits(nc)
    return nc


_NC = None
_W = None


def _install_trace_hook():
    """Register the axon NTFF profile hook (test-only; KERNEL_TRACE=1).

    The image's antenv package lacks axon_hooks, so run_bass_kernel_spmd's
    trace path degrades. Replicate the boot shim: drive NRT profiling via
    ctypes into libaxon_pjrt.so and seed sys.modules so bass_utils finds it.
    """
    import contextlib
    import ctypes
    import sys
    import types

    if "antenv.axon_hooks" in sys.modules:
        return
    so_path = "/opt/axon/libaxon_pjrt.so"
    lib = ctypes.CDLL(so_path)
    if not hasattr(lib, "axon_start_nrt_profile"):
        return
    lib.axon_start_nrt_profile.argtypes = [
        ctypes.POINTER(ctypes.c_int64),
        ctypes.c_size_t,
    ]
    lib.axon_start_nrt_profile.restype = ctypes.c_int64
    lib.axon_stop_nrt_profile.argtypes = [ctypes.c_char_p]
    lib.axon_stop_nrt_profile.restype = ctypes.c_int64

    @contextlib.contextmanager
    def _hook(output_dir, device_ids):
        import jax

        jax.devices()
        if device_ids:
            ids = (ctypes.c_int64 * len(device_ids))(*device_ids)
            rc = lib.axon_start_nrt_profile(ids, len(device_ids))
        else:
            rc = lib.axon_start_nrt_profile(None, 0)
        if rc != 0:
            raise RuntimeError(f"axon_start_nrt_profile rc={rc}")
        try:
            yield
        finally:
            n = lib.axon_stop_nrt_profile(str(output_dir).encode())
            print(f"profile: {n} file(s) written to {output_dir}", file=sys.stderr)

    mod = types.ModuleType("antenv.axon_hooks")
    mod.get_axon_ntff_profile_hook = lambda: _hook
    mod.set_axon_ntff_profile_hook = lambda h: None
    sys.modules["antenv.axon_hooks"] = mod

    # The trace path uploads NEFF artifacts to a remote bucket; no-op it.
    import concourse.bass_utils as _bu

    _bu.upload_artifacts = lambda tmpdir: tmpdir


def _to_bf16(arr: np.ndarray) -> np.ndarray:
    """Fast round-to-nearest-even f32 -> bf16 via integer ops."""
    u = np.ascontiguousarray(arr, dtype=np.float32).view(np.uint32)
    rounded = (u + 0x7FFF + ((u >> 16) & 1)) >> 16
    return rounded.astype(np.uint16).view(NP_BF16)


def kernel(x: np.ndarray, noise: np.ndarray) -> np.ndarray:
    global _NC, _W, LAST_EXEC_NS
    if _NC is None:
        _NC = _build()
        _W = _coeff_matrix()

    in_maps = []
    for c in range(NCORES):
        xs = _to_bf16(x[c * NB : (c + 1) * NB]).reshape(1, E)
        ns = _to_bf16(noise[:, c * NB : (c + 1) * NB]).reshape(S, E)
        in_maps.append({"nz": np.concatenate([ns, xs], axis=0), "w": _W})

    trace = bool(os.environ.get("KERNEL_TRACE"))
    if trace:
        _install_trace_hook()
    res = run_bass_kernel_spmd(_NC, in_maps, list(range(NCORES)), trace=trace)
    LAST_EXEC_NS = res.exec_time_ns

    outs = [
        res.results[c]["out"].astype(np.float32).reshape(K, NB, L, D)
        for c in range(NCORES)
    ]
    return np.concatenate(outs, axis=1)


# revision 20
# speedup vs baseline: 1.1909x; 1.1891x over previous
"""VP-SDE Euler-Maruyama forward diffusion on 8 Trainium2 NeuronCores.

Recurrence (per element, 100 steps):
    x_t = a_t * x_{t-1} + b_t * n_t
      a_t = 1 - 0.5 * beta_t * dt
      b_t = sqrt(beta_t * dt)
      beta_t = BETA0 + (t/S) * (BETA1 - BETA0)

Unrolled, the whole trajectory is a small matmul over the step axis:
    x_k = gamma_k * x_0 + sum_{j<k} (b_j * gamma_k / gamma_{j+1}) * n_j
with gamma_k = prod(a_0..a_{k-1}). Stacking x_0 as a 101st input row, every
output step is one column of a [101, 101] coefficient matrix applied to the
[101, E] input block, so the serial scan becomes a PE matmul and the kernel
is purely streaming.

Sharding: data-parallel over the batch dim (64 -> 8 per core). All HBM I/O
is bf16 (inputs pre-cast on host, outputs upcast on host), which halves the
~101 MiB/core of f32 traffic; the tolerance budget easily covers bf16
rounding. Memory-bound: ~53 MiB of HBM traffic per core.
"""

import os

import ml_dtypes
import numpy as np

import concourse.bass as bass
import concourse.mybir as mybir
from concourse.bass_utils import run_bass_kernel_spmd
from concourse.tile import TileContext

S = 100          # diffusion steps
N, L, D = 64, 256, 64
NCORES = 8
NB = N // NCORES           # batch per core
E = NB * L * D             # elements per core (131072)
K = S + 1                  # matmul contraction: 100 noise rows + x0 row
C = 8192                   # elements per DMA chunk (16 KiB/partition bf16)
MM = 512                   # matmul moving free size (one PSUM bank of f32)

BETA0, BETA1 = 0.1, 20.0
DT = 1.0 / S

F32 = mybir.dt.float32
BF16 = mybir.dt.bfloat16
NP_BF16 = ml_dtypes.bfloat16

LAST_EXEC_NS = None


def _coeff_matrix() -> np.ndarray:
    """W[j, k] = coefficient of input row j in output step k (fp64 -> bf16).

    Rows 0..99 are the noise steps, row 100 is x_0. Column k is output step k
    (k=0 is the initial state itself).
    """
    beta = BETA0 + (np.arange(S, dtype=np.float64) / S) * (BETA1 - BETA0)
    a = 1.0 - 0.5 * beta * DT
    b = np.sqrt(beta * DT)
    gamma = np.concatenate([[1.0], np.cumprod(a)])  # gamma[k] = prod(a[:k])
    W = np.zeros((K, K), dtype=np.float64)
    W[S, :] = gamma                                  # x0 coefficient
    for j in range(S):
        W[j, j + 1 :] = b[j] * gamma[j + 1 :] / gamma[j + 1]
    return W.astype(NP_BF16)


def _legalize_waits(nc, max_waits=1):
    """Split multi-sem waits into standalone EventSemaphore instructions.

    TRN2 TPB instruction encodings carry a single sem-wait slot; walrus
    rejects instructions with more ("Too many sync wait commands"). Tile
    emits up to 3 waits per instruction, so peel the excess onto
    same-engine EventSemaphore instructions placed immediately before —
    engine-queue program order makes this exactly equivalent.
    """
    split_types = tuple(
        t
        for t in (
            getattr(mybir, n, None)
            for n in (
                "InstTensorTensor",
                "InstActivation",
                "InstDMACopy",
                "InstTensorScalarPtr",
                "InstMemset",
                "InstTensorCopy",
                "InstTensorReduce",
                "InstCopy",
                "InstDrain",
                "InstMatmult",
                "InstLdweights",
            )
        )
        if t is not None
    )
    n = 0
    for fn in nc.m.functions:
        for blk in fn.blocks:
            out = []
            for inst in blk.instructions:
                si = inst.sync_info
                if (
                    si is not None
                    and si.on_wait
                    and len(si.on_wait) > max_waits
                    and isinstance(inst, split_types)
                ):
                    for w in si.on_wait[:-max_waits]:
                        n += 1
                        es = mybir.InstEventSemaphore(
                            name=f"legalize-wait-{n}", ins=[], outs=[]
                        )
                        es.name = f"legalize-wait-{n}"
                        es.engine = inst.engine
                        es.sync_info = mybir.SyncInfo(on_wait=[w], on_update=[])
                        nc.register_instruction(es)
                        out.append(es)
                    inst.sync_info = mybir.SyncInfo(
                        on_wait=list(si.on_wait[-max_waits:]),
                        on_update=list(si.on_update or []),
                    )
                out.append(inst)
            blk.instructions = out


def _build():
    nc = bass.Bass()
    nz = nc.declare_dram_parameter("nz", [K, E], BF16, isOutput=False)
    w = nc.declare_dram_parameter("w", [K, K], BF16, isOutput=False)
    out = nc.declare_dram_parameter("out", [K, E], BF16, isOutput=True)

    # HWDGE fans a DMA's partition lines over g SDMA engines where g is the
    # largest divisor of the partition count <= 16. 101 is prime -> g=1, the
    # whole transfer serializes on one engine at ~27 GB/s. Split every
    # 101-partition transfer into 96 + 5 rows (16 engines x 6 lines + 5
    # engines x 1 line) to keep all 16 engines fed.
    PS = 96

    with TileContext(nc) as tc:
        with (
            tc.tile_pool(name="wpool", bufs=1) as wpool,
            tc.tile_pool(name="npool", bufs=3) as npool,
            tc.tile_pool(name="opool", bufs=3) as opool,
            tc.tile_pool(name="pspool", bufs=8, space="PSUM") as pspool,
        ):
            wt = wpool.tile([K, K], BF16)
            nc.sync.dma_start(out=wt[:PS, :], in_=w[:PS, :])
            nc.sync.dma_start(out=wt[PS:, :], in_=w[PS:, :])
            for ch in range(E // C):
                nt = npool.tile([K, C], BF16)
                csl = slice(ch * C, (ch + 1) * C)
                nc.sync.dma_start(out=nt[:PS, :], in_=nz[:PS, csl])
                nc.gpsimd.dma_start(out=nt[PS:, :], in_=nz[PS:, csl])
                ot = opool.tile([K, C], BF16)
                for j in range(C // MM):
                    ps = pspool.tile([K, MM], F32)
                    nc.tensor.matmul(
                        out=ps[:],
                        lhsT=wt[:],
                        rhs=nt[:, j * MM : (j + 1) * MM],
                        start=True,
                        stop=True,
                    )
                    oslc = ot[:, j * MM : (j + 1) * MM]
                    if j % 2 == 0:
                        nc.vector.tensor_copy(oslc, ps[:])
                    else:
                        nc.scalar.copy(oslc, ps[:])
                nc.scalar.dma_start(out=out[:PS, csl], in_=ot[:PS, :])
                nc.gpsimd.dma_start(out=out[PS:, csl], in_=ot[PS:, :])
    _legalize_waits(nc)
    return nc


_NC = None
_W = None


def _install_trace_hook():
    """Register the axon NTFF profile hook (test-only; KERNEL_TRACE=1).

    The image's antenv package lacks axon_hooks, so run_bass_kernel_spmd's
    trace path degrades. Replicate the boot shim: drive NRT profiling via
    ctypes into libaxon_pjrt.so and seed sys.modules so bass_utils finds it.
    """
    import contextlib
    import ctypes
    import sys
    import types

    if "antenv.axon_hooks" in sys.modules:
        return
    so_path = "/opt/axon/libaxon_pjrt.so"
    lib = ctypes.CDLL(so_path)
    if not hasattr(lib, "axon_start_nrt_profile"):
        return
    lib.axon_start_nrt_profile.argtypes = [
        ctypes.POINTER(ctypes.c_int64),
        ctypes.c_size_t,
    ]
    lib.axon_start_nrt_profile.restype = ctypes.c_int64
    lib.axon_stop_nrt_profile.argtypes = [ctypes.c_char_p]
    lib.axon_stop_nrt_profile.restype = ctypes.c_int64

    @contextlib.contextmanager
    def _hook(output_dir, device_ids):
        import jax

        jax.devices()
        if device_ids:
            ids = (ctypes.c_int64 * len(device_ids))(*device_ids)
            rc = lib.axon_start_nrt_profile(ids, len(device_ids))
        else:
            rc = lib.axon_start_nrt_profile(None, 0)
        if rc != 0:
            raise RuntimeError(f"axon_start_nrt_profile rc={rc}")
        try:
            yield
        finally:
            n = lib.axon_stop_nrt_profile(str(output_dir).encode())
            print(f"profile: {n} file(s) written to {output_dir}", file=sys.stderr)

    mod = types.ModuleType("antenv.axon_hooks")
    mod.get_axon_ntff_profile_hook = lambda: _hook
    mod.set_axon_ntff_profile_hook = lambda h: None
    sys.modules["antenv.axon_hooks"] = mod

    # The trace path uploads NEFF artifacts to a remote bucket; no-op it.
    import concourse.bass_utils as _bu

    _bu.upload_artifacts = lambda tmpdir: tmpdir


def _to_bf16(arr: np.ndarray) -> np.ndarray:
    """Fast round-to-nearest-even f32 -> bf16 via integer ops."""
    u = np.ascontiguousarray(arr, dtype=np.float32).view(np.uint32)
    rounded = (u + 0x7FFF + ((u >> 16) & 1)) >> 16
    return rounded.astype(np.uint16).view(NP_BF16)


def kernel(x: np.ndarray, noise: np.ndarray) -> np.ndarray:
    global _NC, _W, LAST_EXEC_NS
    if _NC is None:
        _NC = _build()
        _W = _coeff_matrix()

    in_maps = []
    for c in range(NCORES):
        xs = _to_bf16(x[c * NB : (c + 1) * NB]).reshape(1, E)
        ns = _to_bf16(noise[:, c * NB : (c + 1) * NB]).reshape(S, E)
        in_maps.append({"nz": np.concatenate([ns, xs], axis=0), "w": _W})

    trace = bool(os.environ.get("KERNEL_TRACE"))
    if trace:
        _install_trace_hook()
    res = run_bass_kernel_spmd(_NC, in_maps, list(range(NCORES)), trace=trace)
    LAST_EXEC_NS = res.exec_time_ns

    outs = [
        res.results[c]["out"].astype(np.float32).reshape(K, NB, L, D)
        for c in range(NCORES)
    ]
    return np.concatenate(outs, axis=1)
